# revision 21
# baseline (speedup 1.0000x reference)
"""AttentionSimilarity Trainium2 kernel (8-core SPMD, single fused launch).

Host<->device traffic over the axon tunnel (~44 MB/s) dominates wall time,
so everything runs in ONE launch with minimal wire bytes:
  - features shipped bf16, fully sharded: core c gets its 16 a-batches and
    16 b-batches as an E-major [C, 1568] slab (2.35 MB/core).
  - projector weights shipped SHARDED 1/8 per core (0.5 MB/core) and
    AllGather-ed on device to the full 4 MB.
  - projections q/k/v computed on device; (qa, ka, va) AllGather-ed across
    cores (441 KB/core in, 3.5 MB out) so each core holds the full "a" side.
  - Gram matrices, norms, attention (exp-trick: softmax normalization
    cancels in cosine), cosine, and the mean over q all happen on device.
  - output per core: [128, 32] f32 (16 KB).

Attention structure per direction (from the two-launch baseline):
  scoresT = k_pair.T @ q            (PE)  e = exp(scale*scoresT)   (ACT)
  G = v_pair.T @ v_hat_other        (PE)  num = mask.T @ (e*G)     (PE)
  R = Gram_blockdiag.T @ e          (PE)  den = mask.T @ (e*R)     (PE)
  cos = num / sqrt(den)             (ACT+DVE, v_hat pre-normalized)
"""

import math

import ml_dtypes
import numpy as np

import concourse.bass as bass
from concourse import bacc
import concourse.mybir as mybir
from concourse.tile import TileContext
from concourse.bass_utils import run_bass_kernel_spmd

BF16 = mybir.dt.bfloat16
F32 = mybir.dt.float32
FP8 = mybir.dt.float8e4
NPBF = ml_dtypes.bfloat16
NPF8 = ml_dtypes.float8_e4m3

B = 128
C = 768
S = 49
E = 96
NCORES = 8
BL = B // NCORES          # 16 local batches
NL = BL * S               # 784 local rows
NROWS = 2 * NL            # 1568 rows per core (a slab then b slab)
KT = C // 128             # 6 contraction tiles
W1E = 3 * C * C           # 1769472
W2E = 3 * C * E           # 221184
WELEM = W1E + W2E         # 1990656
WSH = WELEM // NCORES     # 248832
SCALE = 1.0 / math.sqrt(E)
GRP = [list(range(NCORES))]

TRACE = False
LAST_EXEC_NS = [None, None]

_CACHE = {}


def _nchunks(total, step=512):
    out = []
    n0 = 0
    while n0 < total:
        out.append((n0, min(step, total - n0)))
        n0 += step
    return out


def _phase_b_projections(nc, tc, x, wg, ptp):
    """pT[e, w, n] = ([relu(x.T @ W1_w) @ W2_w]).T, f32 in SBUF.

    x: fp8 AP [C*NROWS]; wg: fp8 byte AP [2*WELEM] holding bf16 weights.
    """
    RELU = mybir.ActivationFunctionType.Relu
    pT = ptp.tile([E, 3, NROWS], F32, tag="pT")
    with (
        tc.tile_pool(name="xp", bufs=1) as xp,
        tc.tile_pool(name="xcp", bufs=2) as xcp,
        tc.tile_pool(name="wp", bufs=1) as wp,
        tc.tile_pool(name="hp", bufs=1) as hp,
        tc.tile_pool(name="pp1", bufs=4, space="PSUM") as pp1,
        tc.tile_pool(name="pp2", bufs=2, space="PSUM") as pp2,
    ):
        x_sb = xp.tile([128, KT, NROWS], FP8)
        nc.sync.dma_start(
            out=x_sb, in_=x.rearrange("(t p) n -> p t n", p=128)
        )
        w1_sb = wp.tile([128, 3, KT, C], BF16, tag="w1")
        nc.sync.dma_start(
            out=w1_sb,
            in_=wg[0:W1E].rearrange(
                "(w t p n) -> p w t n", w=3, t=KT, p=128, n=C
            ),
        )
        w2_sb = wp.tile([128, 3, KT, E], BF16, tag="w2")
        nc.sync.dma_start(
            out=w2_sb,
            in_=wg[W1E:WELEM].rearrange(
                "(w t p n) -> p w t n", w=3, t=KT, p=128, n=E
            ),
        )

        for w in range(3):
            hT = hp.tile([128, KT, NROWS], BF16, tag="hT")
            for n0, nsz in _nchunks(NROWS):
                xc = xcp.tile([128, KT, 512], BF16, tag="xc")
                nc.scalar.copy(xc[:, :, :nsz], x_sb[:, :, n0:n0 + nsz])
                for m in range(KT):
                    ps = pp1.tile([128, 512], F32, tag="ps1")
                    for k in range(KT):
                        nc.tensor.matmul(
                            ps[:, :nsz],
                            lhsT=w1_sb[:, w, k, m * 128:(m + 1) * 128],
                            rhs=xc[:, k, :nsz],
                            start=(k == 0),
                            stop=(k == KT - 1),
                        )
                    nc.scalar.activation(hT[:, m, n0:n0 + nsz], ps[:, :nsz], RELU)
            for n0, nsz in _nchunks(NROWS):
                ps2 = pp2.tile([E, 512], F32, tag="ps2")
                for k in range(KT):
                    nc.tensor.matmul(
                        ps2[:, :nsz],
                        lhsT=w2_sb[:, w, k, :],
                        rhs=hT[:, k, n0:n0 + nsz],
                        start=(k == 0),
                        stop=(k == KT - 1),
                    )
                nc.scalar.copy(pT[:, w, n0:n0 + nsz], ps2[:, :nsz])
    return pT


XB = C * NROWS            # fp8 feature bytes per core
WB = 2 * WSH              # weight-shard bytes per core (bf16 as raw bytes)


def _build_nc():
    nc = bacc.Bacc(target_bir_lowering=False, num_devices=NCORES)
    xp_ = nc.declare_dram_parameter("x", [C, NROWS], FP8, isOutput=False)
    wsh = nc.declare_dram_parameter("wsh", [WSH], BF16, isOutput=False)
    osim = nc.declare_dram_parameter("osim", [128, 32], F32, isOutput=True)

    EXP = mybir.ActivationFunctionType.Exp
    SQRT = mybir.ActivationFunctionType.Sqrt
    BYP = mybir.AluOpType.bypass

    with TileContext(nc) as tc:
        with (
            tc.tile_pool(name="dram", bufs=1, space="DRAM") as dp,
            tc.tile_pool(name="cst", bufs=1) as cst,
        ):
            # ---- Phase A: weight shard AllGather (overlaps x load) ----
            wsh_b = dp.tile([WSH], BF16, tag="wshb")
            wg = dp.tile([WELEM], BF16, tag="wg")
            nc.gpsimd.dma_start(out=wsh_b, in_=wsh[:])
            nc.gpsimd.collective_compute(
                "AllGather", BYP, replica_groups=GRP,
                ins=[wsh_b[:].opt()], outs=[wg[:].opt()],
            )

            # persistent attention operands
            qb_sb = cst.tile([E, NL], BF16, tag="qb")
            vbh_sb = cst.tile([E, NL], BF16, tag="vbh")
            kbp_sb = cst.tile([E, 8, 128], BF16, tag="kbp")
            vbp_sb = cst.tile([E, 8, 128], BF16, tag="vbp")
            qa_sb = cst.tile([E, 8 * NL], BF16, tag="qa")
            vah_sb = cst.tile([E, 8 * NL], BF16, tag="vah")
            kap_sb = cst.tile([E, 64, 128], BF16, tag="kap")
            vap_sb = cst.tile([E, 64, 128], BF16, tag="vap")
            ma_sb = cst.tile([128, 64, 128], BF16, tag="ma")
            mb_sb = cst.tile([128, 8, 128], BF16, tag="mb")
            msk_sb = cst.tile([128, 256], BF16, tag="msk")
            cos_sb = cst.tile([128, 2, NL], F32, tag="cos")
            ones_c = cst.tile([E, 1], F32, tag="onec")
            ones_r = cst.tile([1, E], F32, tag="oner")

            nc.vector.memset(msk_sb[:], 0.0)
            nc.vector.memset(msk_sb[0:S, 126:127], 1.0)
            nc.vector.memset(msk_sb[64:64 + S, 127:128], 1.0)
            nc.vector.memset(ones_c[:], 1.0)
            nc.vector.memset(ones_r[:], 1.0)
            nc.vector.memset(kbp_sb[:], 0.0)
            nc.vector.memset(vbp_sb[:], 0.0)
            nc.vector.memset(kap_sb[:], 0.0)
            nc.vector.memset(vap_sb[:], 0.0)

            with tc.tile_pool(name="ptp", bufs=1) as ptp:
                # ---- Phase B: projections ----
                pT = _phase_b_projections(nc, tc, xp_, wg, ptp)

                # ---- Phase C: gather (qa,ka,va) + local bf16 prep ----
                pg_sb = ptp.tile([E, 3, NL], BF16, tag="pg")
                for w in range(3):
                    nc.scalar.copy(pg_sb[:, w, :], pT[:, w, 0:NL])
                g_in = dp.tile([E, 3, NL], BF16, tag="gin")
                gout = dp.tile([NCORES * E * 3 * NL], BF16, tag="gout")
                nc.sync.dma_start(out=g_in, in_=pg_sb)
                nc.gpsimd.collective_compute(
                    "AllGather", BYP, replica_groups=GRP,
                    ins=[g_in[:].opt()], outs=[gout[:].opt()],
                )

                nc.scalar.copy(qb_sb[:], pT[:, 0, NL:NROWS])
                for w, dst in ((1, kbp_sb), (2, vbp_sb)):
                    src = pT[:, w, NL:NROWS].rearrange(
                        "p (b2 i s) -> p i b2 s", b2=8, i=2, s=S
                    )
                    for i in range(2):
                        nc.scalar.copy(dst[:, :, 64 * i:64 * i + S], src[:, i])

                with (
                    tc.tile_pool(name="np1", bufs=2) as np1,
                    tc.tile_pool(name="npp", bufs=2, space="PSUM") as npp,
                ):
                    def normalize(dst_ap, src_ap, nsz):
                        sq = np1.tile([E, 512], F32, tag="sq")
                        nc.vector.tensor_mul(sq[:, :nsz], src_ap, src_ap)
                        ssq = npp.tile([1, 512], F32, tag="ssq")
                        nc.tensor.matmul(
                            ssq[:, :nsz], lhsT=ones_c[:, :], rhs=sq[:, :nsz],
                            start=True, stop=True,
                        )
                        rno = np1.tile([1, 512], F32, tag="rno")
                        nc.scalar.activation(rno[:, :nsz], ssq[:, :nsz], SQRT)
                        rrec = np1.tile([1, 512], F32, tag="rrec")
                        nc.vector.reciprocal(rrec[:, :nsz], rno[:, :nsz])
                        bc = npp.tile([E, 512], F32, tag="bc")
                        nc.tensor.matmul(
                            bc[:, :nsz], lhsT=ones_r[:, :], rhs=rrec[:, :nsz],
                            start=True, stop=True,
                        )
                        nc.vector.tensor_mul(dst_ap, src_ap, bc[:, :nsz])

                    # vb_hat from local f32 vb
                    for n0, nsz in _nchunks(NL):
                        normalize(
                            vbh_sb[:, n0:n0 + nsz],
                            pT[:, 2, NL + n0:NL + n0 + nsz], nsz,
                        )

                    # gathered loads
                    gv = gout.rearrange(
                        "(c p w n) -> p w c n", c=NCORES, p=E, w=3, n=NL
                    )
                    nc.sync.dma_start(
                        out=qa_sb.rearrange("p (c n) -> p c n", c=NCORES, n=NL),
                        in_=gv[:, 0],
                    )
                    va_fl = ptp.tile([E, 8 * NL], BF16, tag="vafl")
                    nc.sync.dma_start(
                        out=va_fl.rearrange("p (c n) -> p c n", c=NCORES, n=NL),
                        in_=gv[:, 2],
                    )
                    gvp = gout.rearrange(
                        "(c p w b2 i s) -> p w i c b2 s",
                        c=NCORES, p=E, w=3, b2=8, i=2, s=S,
                    )
                    for i in range(2):
                        for cc in range(NCORES):
                            nc.sync.dma_start(
                                out=kap_sb[:, cc * 8:(cc + 1) * 8,
                                           64 * i:64 * i + S],
                                in_=gvp[:, 1, i, cc],
                            )
                            nc.sync.dma_start(
                                out=vap_sb[:, cc * 8:(cc + 1) * 8,
                                           64 * i:64 * i + S],
                                in_=gvp[:, 2, i, cc],
                            )

                    # va_hat from gathered bf16 va
                    for n0, nsz in _nchunks(8 * NL):
                        normalize(
                            vah_sb[:, n0:n0 + nsz], va_fl[:, n0:n0 + nsz], nsz
                        )

                # Gram matrices (blockdiag pair layout), from bf16 pads
                with tc.tile_pool(name="grm", bufs=4, space="PSUM") as grm:
                    nc.vector.memset(ma_sb[:], 0.0)
                    nc.vector.memset(mb_sb[:], 0.0)
                    for j in range(64):
                        pg = grm.tile([128, 128], F32, tag="g")
                        for i in range(2):
                            sl = slice(64 * i, 64 * i + S)
                            nc.tensor.matmul(
                                pg[sl, sl],
                                lhsT=vap_sb[:, j, sl], rhs=vap_sb[:, j, sl],
                                start=True, stop=True,
                            )
                            nc.scalar.copy(ma_sb[sl, j, sl], pg[sl, sl])
                    for p8 in range(8):
                        pg = grm.tile([128, 128], F32, tag="g")
                        for i in range(2):
                            sl = slice(64 * i, 64 * i + S)
                            nc.tensor.matmul(
                                pg[sl, sl],
                                lhsT=vbp_sb[:, p8, sl], rhs=vbp_sb[:, p8, sl],
                                start=True, stop=True,
                            )
                            nc.scalar.copy(mb_sb[sl, p8, sl], pg[sl, sl])

            # ---- Phase D: attention + cosine + q-sum ----
            with (
                tc.tile_pool(name="ep", bufs=6) as ep,
                tc.tile_pool(name="prp", bufs=6) as prp,
                tc.tile_pool(name="ep2", bufs=2) as ep2,
                tc.tile_pool(name="op", bufs=1) as op,
                tc.tile_pool(name="sgr", bufs=2, space="PSUM") as sgr,
                tc.tile_pool(name="grp", bufs=2, space="PSUM") as grp_ps,
                tc.tile_pool(name="ppd", bufs=1, space="PSUM") as ppd,
            ):
                chunks = _nchunks(NL)
                for d in range(2):
                    if d == 0:  # dir ba: a-pair j vs all local b
                        units = [
                            (
                                kap_sb[:, j, :],
                                vap_sb[:, j, :],
                                qb_sb,
                                vbh_sb,
                                ma_sb[:, j, :],
                            )
                            for j in range(64)
                        ]
                    else:  # dir ab: local b-pair p vs a-chunk cch
                        units = [
                            (
                                kbp_sb[:, p, :],
                                vbp_sb[:, p, :],
                                qa_sb[:, cch * NL:(cch + 1) * NL],
                                vah_sb[:, cch * NL:(cch + 1) * NL],
                                mb_sb[:, p, :],
                            )
                            for p in range(8)
                            for cch in range(8)
                        ]
                    for n0, nsz in chunks:
                        ps_num = ppd.tile([128, 512], F32, tag="dnum")
                        ps_den = ppd.tile([128, 512], F32, tag="dden")
                        for j, (lk, lv, rq, rv, mm) in enumerate(units):
                            mwin = msk_sb[:, 126 - 2 * j:254 - 2 * j]
                            ps_s = sgr.tile([128, 512], F32, tag="sgr")
                            nc.tensor.matmul(
                                ps_s[:, :nsz],
                                lhsT=lk,
                                rhs=rq[:, n0:n0 + nsz],
                                start=True,
                                stop=True,
                            )
                            eh = ep.tile([128, 512], BF16, tag="eh")
                            nc.scalar.activation(
                                eh[:, :nsz], ps_s[:, :nsz], EXP, scale=SCALE
                            )
                            ps_gr = grp_ps.tile([128, 2, 512], F32, tag="gr2")
                            nc.tensor.matmul(
                                ps_gr[:, 0, :nsz],
                                lhsT=lv,
                                rhs=rv[:, n0:n0 + nsz],
                                start=True,
                                stop=True,
                            )
                            nc.tensor.matmul(
                                ps_gr[:, 1, :nsz],
                                lhsT=mm,
                                rhs=eh[:, :nsz],
                                start=True,
                                stop=True,
                            )
                            pgr = prp.tile([128, 2, 512], BF16, tag="pgr")
                            eh2 = bass.AP(
                                tensor=eh.tensor,
                                offset=eh.offset,
                                ap=[eh.ap[0], [0, 2], [1, nsz]],
                            )
                            nc.vector.tensor_mul(
                                pgr[:, :, :nsz], eh2, ps_gr[:, :, :nsz]
                            )
                            nc.tensor.matmul(
                                ps_num[:, :nsz],
                                lhsT=mwin,
                                rhs=pgr[:, 0, :nsz],
                                start=(j == 0),
                                stop=(j == 63),
                            )
                            nc.tensor.matmul(
                                ps_den[:, :nsz],
                                lhsT=mwin,
                                rhs=pgr[:, 1, :nsz],
                                start=(j == 0),
                                stop=(j == 63),
                            )
                        sq_d = ep2.tile([128, 512], F32, tag="sqd")
                        nc.scalar.activation(sq_d[:, :nsz], ps_den[:, :nsz], SQRT)
                        rc_d = ep2.tile([128, 512], F32, tag="rcd")
                        nc.vector.reciprocal(rc_d[:, :nsz], sq_d[:, :nsz])
                        nc.vector.tensor_mul(
                            cos_sb[:, d, n0:n0 + nsz],
                            ps_num[:, :nsz],
                            rc_d[:, :nsz],
                        )

                red_sb = op.tile([128, 32], F32, tag="red")
                nc.vector.reduce_sum(
                    red_sb[:, :],
                    cos_sb.rearrange("p d (b s) -> p d b s", b=BL, s=S),
                    axis=mybir.AxisListType.X,
                )
                nc.sync.dma_start(out=osim[:, :], in_=red_sb)
    if not nc.is_finalized():
        nc.finalize()
    return nc


def kernel(features_a, features_b, Wq1, Wq2, Wk1, Wk2, Wv1, Wv2):
    import time as _t

    features_a = np.asarray(features_a, dtype=np.float32)
    features_b = np.asarray(features_b, dtype=np.float32)
    fa = np.ascontiguousarray(features_a.reshape(B, C, S))
    fb = np.ascontiguousarray(features_b.reshape(B, C, S))

    if "nc" not in _CACHE:
        _CACHE["nc"] = _build_nc()

    w1 = np.stack([Wq1, Wk1, Wv1]).astype(np.float32).astype(NPBF).reshape(-1)
    w2 = np.stack([Wq2, Wk2, Wv2]).astype(np.float32).astype(NPBF).reshape(-1)
    wflat = np.concatenate([w1, w2])

    in_maps = []
    for c in range(NCORES):
        sl = slice(c * BL, (c + 1) * BL)
        xa = fa[sl].transpose(1, 0, 2).reshape(C, NL)
        xb = fb[sl].transpose(1, 0, 2).reshape(C, NL)
        xT = np.concatenate([xa, xb], axis=1).astype(NPF8)
        in_maps.append({"x": xT, "wsh": wflat[c * WSH:(c + 1) * WSH]})

    t0 = _t.time()
    res = run_bass_kernel_spmd(
        _CACHE["nc"], in_maps, list(range(NCORES)), trace=TRACE
    )
    LAST_EXEC_NS[0] = int((_t.time() - t0) * 1e9)

    sim = np.zeros((B, B), dtype=np.float64)
    for c in range(NCORES):
        o = res.results[c]["osim"].astype(np.float64).reshape(128, 2, BL)
        bidx = slice(c * BL, (c + 1) * BL)
        # dir ba: rows = global a, cols = local b
        sim[bidx, :] += o[:, 0, :].T
        # dir ab: rows = (p, cch, i), cols = aloc; b_local = 2p+i, a = cch*16+aloc
        ab = o[:, 1, :].reshape(8, 8, 2, BL)
        sim[bidx, :] += ab.transpose(0, 2, 1, 3).reshape(BL, B)
    return (sim / S).astype(np.float32)


# revision 22
# speedup vs baseline: 1.3178x; 1.3178x over previous
"""AttentionSimilarity Trainium2 kernel (8-core SPMD, single fused launch).

Host<->device traffic over the axon tunnel (~44 MB/s) dominates wall time,
so everything runs in ONE launch with minimal wire bytes:
  - features shipped bf16, fully sharded: core c gets its 16 a-batches and
    16 b-batches as an E-major [C, 1568] slab (2.35 MB/core).
  - projector weights shipped SHARDED 1/8 per core (0.5 MB/core) and
    AllGather-ed on device to the full 4 MB.
  - projections q/k/v computed on device; (qa, ka, va) AllGather-ed across
    cores (441 KB/core in, 3.5 MB out) so each core holds the full "a" side.
  - Gram matrices, norms, attention (exp-trick: softmax normalization
    cancels in cosine), cosine, and the mean over q all happen on device.
  - output per core: [128, 32] f32 (16 KB).

Attention structure per direction (from the two-launch baseline):
  scoresT = k_pair.T @ q            (PE)  e = exp(scale*scoresT)   (ACT)
  G = v_pair.T @ v_hat_other        (PE)  num = mask.T @ (e*G)     (PE)
  R = Gram_blockdiag.T @ e          (PE)  den = mask.T @ (e*R)     (PE)
  cos = num / sqrt(den)             (ACT+DVE, v_hat pre-normalized)
"""

import math

import ml_dtypes
import numpy as np

import concourse.bass as bass
from concourse import bacc
import concourse.mybir as mybir
from concourse.tile import TileContext
from concourse.bass_utils import run_bass_kernel_spmd

BF16 = mybir.dt.bfloat16
F32 = mybir.dt.float32
FP8 = mybir.dt.float8e4
NPBF = ml_dtypes.bfloat16
NPF8 = ml_dtypes.float8_e4m3

B = 128
C = 768
S = 49
E = 96
NCORES = 8
BL = B // NCORES          # 16 local batches
NL = BL * S               # 784 local rows
NROWS = 2 * NL            # 1568 rows per core (a slab then b slab)
KT = C // 128             # 6 contraction tiles
W1E = 3 * C * C           # 1769472
W2E = 3 * C * E           # 221184
WELEM = W1E + W2E         # 1990656
WSH = WELEM // NCORES     # 248832
SCALE = 1.0 / math.sqrt(E)
GRP = [list(range(NCORES))]

TRACE = False
LAST_EXEC_NS = [None, None]

_CACHE = {}


def _nchunks(total, step=512):
    out = []
    n0 = 0
    while n0 < total:
        out.append((n0, min(step, total - n0)))
        n0 += step
    return out


def _phase_b_projections(nc, tc, x, wg, ptp):
    """pT[e, w, n] = ([relu(x.T @ W1_w) @ W2_w]).T, f32 in SBUF.

    x: fp8 AP [C*NROWS]; wg: fp8 byte AP [2*WELEM] holding bf16 weights.
    """
    RELU = mybir.ActivationFunctionType.Relu
    pT = ptp.tile([E, 3, NROWS], F32, tag="pT")
    with (
        tc.tile_pool(name="xp", bufs=1) as xp,
        tc.tile_pool(name="xcp", bufs=2) as xcp,
        tc.tile_pool(name="wp", bufs=1) as wp,
        tc.tile_pool(name="hp", bufs=1) as hp,
        tc.tile_pool(name="pp1", bufs=4, space="PSUM") as pp1,
        tc.tile_pool(name="pp2", bufs=2, space="PSUM") as pp2,
    ):
        x_sb = xp.tile([128, KT, NROWS], FP8)
        nc.sync.dma_start(
            out=x_sb, in_=x.rearrange("(t p) n -> p t n", p=128)
        )
        w1_sb = wp.tile([128, 3, KT, C], BF16, tag="w1")
        nc.sync.dma_start(
            out=w1_sb,
            in_=wg[0:2 * W1E].bitcast(BF16).rearrange(
                "(w t p n) -> p w t n", w=3, t=KT, p=128, n=C
            ),
        )
        w2_sb = wp.tile([128, 3, KT, E], BF16, tag="w2")
        nc.sync.dma_start(
            out=w2_sb,
            in_=wg[2 * W1E:2 * WELEM].bitcast(BF16).rearrange(
                "(w t p n) -> p w t n", w=3, t=KT, p=128, n=E
            ),
        )

        for w in range(3):
            hT = hp.tile([128, KT, NROWS], BF16, tag="hT")
            for n0, nsz in _nchunks(NROWS):
                xc = xcp.tile([128, KT, 512], BF16, tag="xc")
                nc.scalar.copy(xc[:, :, :nsz], x_sb[:, :, n0:n0 + nsz])
                for m in range(KT):
                    ps = pp1.tile([128, 512], F32, tag="ps1")
                    for k in range(KT):
                        nc.tensor.matmul(
                            ps[:, :nsz],
                            lhsT=w1_sb[:, w, k, m * 128:(m + 1) * 128],
                            rhs=xc[:, k, :nsz],
                            start=(k == 0),
                            stop=(k == KT - 1),
                        )
                    nc.scalar.activation(hT[:, m, n0:n0 + nsz], ps[:, :nsz], RELU)
            for n0, nsz in _nchunks(NROWS):
                ps2 = pp2.tile([E, 512], F32, tag="ps2")
                for k in range(KT):
                    nc.tensor.matmul(
                        ps2[:, :nsz],
                        lhsT=w2_sb[:, w, k, :],
                        rhs=hT[:, k, n0:n0 + nsz],
                        start=(k == 0),
                        stop=(k == KT - 1),
                    )
                nc.scalar.copy(pT[:, w, n0:n0 + nsz], ps2[:, :nsz])
    return pT


XB = C * NROWS            # fp8 feature bytes per core
WB = 2 * WSH              # weight-shard bytes per core (bf16 as raw bytes)


def _build_nc():
    nc = bacc.Bacc(target_bir_lowering=False, num_devices=NCORES)
    xp_ = nc.declare_dram_parameter("x", [C, NROWS], FP8, isOutput=False)
    wsh = nc.declare_dram_parameter("wsh", [WB], FP8, isOutput=False)
    osim = nc.declare_dram_parameter("osim", [128, 32], F32, isOutput=True)

    EXP = mybir.ActivationFunctionType.Exp
    SQRT = mybir.ActivationFunctionType.Sqrt
    BYP = mybir.AluOpType.bypass

    with TileContext(nc) as tc:
        with (
            tc.tile_pool(name="dram", bufs=1, space="DRAM") as dp,
            tc.tile_pool(name="cst", bufs=1) as cst,
        ):
            # ---- Phase A: weight shard AllGather (overlaps x load) ----
            wsh_b = dp.tile([WB], FP8, tag="wshb")
            wg = dp.tile([NCORES * WB], FP8, tag="wg")
            nc.gpsimd.dma_start(out=wsh_b, in_=wsh[:])
            nc.gpsimd.collective_compute(
                "AllGather", BYP, replica_groups=GRP,
                ins=[wsh_b[:].opt()], outs=[wg[:].opt()],
            )

            # persistent attention operands
            qb_sb = cst.tile([E, NL], BF16, tag="qb")
            vbh_sb = cst.tile([E, NL], BF16, tag="vbh")
            kbp_sb = cst.tile([E, 8, 128], BF16, tag="kbp")
            vbp_sb = cst.tile([E, 8, 128], BF16, tag="vbp")
            qa_sb = cst.tile([E, 8 * NL], BF16, tag="qa")
            vah_sb = cst.tile([E, 8 * NL], BF16, tag="vah")
            kap_sb = cst.tile([E, 64, 128], BF16, tag="kap")
            vap_sb = cst.tile([E, 64, 128], BF16, tag="vap")
            ma_sb = cst.tile([128, 64, 128], BF16, tag="ma")
            mb_sb = cst.tile([128, 8, 128], BF16, tag="mb")
            msk_sb = cst.tile([128, 256], BF16, tag="msk")
            cos_sb = cst.tile([128, 2, NL], F32, tag="cos")
            ones_c = cst.tile([E, 1], F32, tag="onec")
            ones_r = cst.tile([1, E], F32, tag="oner")

            nc.vector.memset(msk_sb[:], 0.0)
            nc.vector.memset(msk_sb[0:S, 126:127], 1.0)
            nc.vector.memset(msk_sb[64:64 + S, 127:128], 1.0)
            nc.vector.memset(ones_c[:], 1.0)
            nc.vector.memset(ones_r[:], 1.0)
            nc.vector.memset(kbp_sb[:], 0.0)
            nc.vector.memset(vbp_sb[:], 0.0)
            nc.vector.memset(kap_sb[:], 0.0)
            nc.vector.memset(vap_sb[:], 0.0)

            with tc.tile_pool(name="ptp", bufs=1) as ptp:
                # ---- Phase B: projections ----
                pT = _phase_b_projections(nc, tc, xp_, wg, ptp)

                # ---- Phase C: gather (qa,ka,va) + local bf16 prep ----
                pg_sb = ptp.tile([E, 3, NL], BF16, tag="pg")
                for w in range(3):
                    nc.scalar.copy(pg_sb[:, w, :], pT[:, w, 0:NL])
                g_in = dp.tile([E, 3, NL], BF16, tag="gin")
                gout = dp.tile([NCORES * E * 3 * NL], BF16, tag="gout")
                nc.sync.dma_start(out=g_in, in_=pg_sb)
                nc.gpsimd.collective_compute(
                    "AllGather", BYP, replica_groups=GRP,
                    ins=[g_in[:].opt()], outs=[gout[:].opt()],
                )

                nc.scalar.copy(qb_sb[:], pT[:, 0, NL:NROWS])
                for w, dst in ((1, kbp_sb), (2, vbp_sb)):
                    src = pT[:, w, NL:NROWS].rearrange(
                        "p (b2 i s) -> p i b2 s", b2=8, i=2, s=S
                    )
                    for i in range(2):
                        nc.scalar.copy(dst[:, :, 64 * i:64 * i + S], src[:, i])

                with (
                    tc.tile_pool(name="np1", bufs=2) as np1,
                    tc.tile_pool(name="npp", bufs=2, space="PSUM") as npp,
                ):
                    def normalize(dst_ap, src_ap, nsz):
                        sq = np1.tile([E, 512], F32, tag="sq")
                        nc.vector.tensor_mul(sq[:, :nsz], src_ap, src_ap)
                        ssq = npp.tile([1, 512], F32, tag="ssq")
                        nc.tensor.matmul(
                            ssq[:, :nsz], lhsT=ones_c[:, :], rhs=sq[:, :nsz],
                            start=True, stop=True,
                        )
                        rno = np1.tile([1, 512], F32, tag="rno")
                        nc.scalar.activation(rno[:, :nsz], ssq[:, :nsz], SQRT)
                        rrec = np1.tile([1, 512], F32, tag="rrec")
                        nc.vector.reciprocal(rrec[:, :nsz], rno[:, :nsz])
                        bc = npp.tile([E, 512], F32, tag="bc")
                        nc.tensor.matmul(
                            bc[:, :nsz], lhsT=ones_r[:, :], rhs=rrec[:, :nsz],
                            start=True, stop=True,
                        )
                        nc.vector.tensor_mul(dst_ap, src_ap, bc[:, :nsz])

                    # vb_hat from local f32 vb
                    for n0, nsz in _nchunks(NL):
                        normalize(
                            vbh_sb[:, n0:n0 + nsz],
                            pT[:, 2, NL + n0:NL + n0 + nsz], nsz,
                        )

                    # gathered loads
                    gv = gout.rearrange(
                        "(c p w n) -> p w c n", c=NCORES, p=E, w=3, n=NL
                    )
                    nc.sync.dma_start(
                        out=qa_sb.rearrange("p (c n) -> p c n", c=NCORES, n=NL),
                        in_=gv[:, 0],
                    )
                    va_fl = ptp.tile([E, 8 * NL], BF16, tag="vafl")
                    nc.sync.dma_start(
                        out=va_fl.rearrange("p (c n) -> p c n", c=NCORES, n=NL),
                        in_=gv[:, 2],
                    )
                    gvp = gout.rearrange(
                        "(c p w b2 i s) -> p w i c b2 s",
                        c=NCORES, p=E, w=3, b2=8, i=2, s=S,
                    )
                    for i in range(2):
                        for cc in range(NCORES):
                            nc.sync.dma_start(
                                out=kap_sb[:, cc * 8:(cc + 1) * 8,
                                           64 * i:64 * i + S],
                                in_=gvp[:, 1, i, cc],
                            )
                            nc.sync.dma_start(
                                out=vap_sb[:, cc * 8:(cc + 1) * 8,
                                           64 * i:64 * i + S],
                                in_=gvp[:, 2, i, cc],
                            )

                    # va_hat from gathered bf16 va
                    for n0, nsz in _nchunks(8 * NL):
                        normalize(
                            vah_sb[:, n0:n0 + nsz], va_fl[:, n0:n0 + nsz], nsz
                        )

                # Gram matrices (blockdiag pair layout), from bf16 pads
                with tc.tile_pool(name="grm", bufs=4, space="PSUM") as grm:
                    nc.vector.memset(ma_sb[:], 0.0)
                    nc.vector.memset(mb_sb[:], 0.0)
                    for j in range(64):
                        pg = grm.tile([128, 128], F32, tag="g")
                        for i in range(2):
                            sl = slice(64 * i, 64 * i + S)
                            nc.tensor.matmul(
                                pg[sl, sl],
                                lhsT=vap_sb[:, j, sl], rhs=vap_sb[:, j, sl],
                                start=True, stop=True,
                            )
                            nc.scalar.copy(ma_sb[sl, j, sl], pg[sl, sl])
                    for p8 in range(8):
                        pg = grm.tile([128, 128], F32, tag="g")
                        for i in range(2):
                            sl = slice(64 * i, 64 * i + S)
                            nc.tensor.matmul(
                                pg[sl, sl],
                                lhsT=vbp_sb[:, p8, sl], rhs=vbp_sb[:, p8, sl],
                                start=True, stop=True,
                            )
                            nc.scalar.copy(mb_sb[sl, p8, sl], pg[sl, sl])

            # ---- Phase D: attention + cosine + q-sum ----
            with (
                tc.tile_pool(name="ep", bufs=6) as ep,
                tc.tile_pool(name="prp", bufs=6) as prp,
                tc.tile_pool(name="ep2", bufs=2) as ep2,
                tc.tile_pool(name="op", bufs=1) as op,
                tc.tile_pool(name="sgr", bufs=2, space="PSUM") as sgr,
                tc.tile_pool(name="grp", bufs=2, space="PSUM") as grp_ps,
                tc.tile_pool(name="ppd", bufs=1, space="PSUM") as ppd,
            ):
                chunks = _nchunks(NL)
                for d in range(2):
                    if d == 0:  # dir ba: a-pair j vs all local b
                        units = [
                            (
                                kap_sb[:, j, :],
                                vap_sb[:, j, :],
                                qb_sb,
                                vbh_sb,
                                ma_sb[:, j, :],
                            )
                            for j in range(64)
                        ]
                    else:  # dir ab: local b-pair p vs a-chunk cch
                        units = [
                            (
                                kbp_sb[:, p, :],
                                vbp_sb[:, p, :],
                                qa_sb[:, cch * NL:(cch + 1) * NL],
                                vah_sb[:, cch * NL:(cch + 1) * NL],
                                mb_sb[:, p, :],
                            )
                            for p in range(8)
                            for cch in range(8)
                        ]
                    for n0, nsz in chunks:
                        ps_num = ppd.tile([128, 512], F32, tag="dnum")
                        ps_den = ppd.tile([128, 512], F32, tag="dden")
                        for j, (lk, lv, rq, rv, mm) in enumerate(units):
                            mwin = msk_sb[:, 126 - 2 * j:254 - 2 * j]
                            ps_s = sgr.tile([128, 512], F32, tag="sgr")
                            nc.tensor.matmul(
                                ps_s[:, :nsz],
                                lhsT=lk,
                                rhs=rq[:, n0:n0 + nsz],
                                start=True,
                                stop=True,
                            )
                            eh = ep.tile([128, 512], BF16, tag="eh")
                            nc.scalar.activation(
                                eh[:, :nsz], ps_s[:, :nsz], EXP, scale=SCALE
                            )
                            ps_gr = grp_ps.tile([128, 2, 512], F32, tag="gr2")
                            nc.tensor.matmul(
                                ps_gr[:, 0, :nsz],
                                lhsT=lv,
                                rhs=rv[:, n0:n0 + nsz],
                                start=True,
                                stop=True,
                            )
                            nc.tensor.matmul(
                                ps_gr[:, 1, :nsz],
                                lhsT=mm,
                                rhs=eh[:, :nsz],
                                start=True,
                                stop=True,
                            )
                            pgr = prp.tile([128, 2, 512], BF16, tag="pgr")
                            eh2 = bass.AP(
                                tensor=eh.tensor,
                                offset=eh.offset,
                                ap=[eh.ap[0], [0, 2], [1, nsz]],
                            )
                            nc.vector.tensor_mul(
                                pgr[:, :, :nsz], eh2, ps_gr[:, :, :nsz]
                            )
                            nc.tensor.matmul(
                                ps_num[:, :nsz],
                                lhsT=mwin,
                                rhs=pgr[:, 0, :nsz],
                                start=(j == 0),
                                stop=(j == 63),
                            )
                            nc.tensor.matmul(
                                ps_den[:, :nsz],
                                lhsT=mwin,
                                rhs=pgr[:, 1, :nsz],
                                start=(j == 0),
                                stop=(j == 63),
                            )
                        sq_d = ep2.tile([128, 512], F32, tag="sqd")
                        nc.scalar.activation(sq_d[:, :nsz], ps_den[:, :nsz], SQRT)
                        rc_d = ep2.tile([128, 512], F32, tag="rcd")
                        nc.vector.reciprocal(rc_d[:, :nsz], sq_d[:, :nsz])
                        nc.vector.tensor_mul(
                            cos_sb[:, d, n0:n0 + nsz],
                            ps_num[:, :nsz],
                            rc_d[:, :nsz],
                        )

                red_sb = op.tile([128, 32], F32, tag="red")
                nc.vector.reduce_sum(
                    red_sb[:, :],
                    cos_sb.rearrange("p d (b s) -> p d b s", b=BL, s=S),
                    axis=mybir.AxisListType.X,
                )
                nc.sync.dma_start(out=osim[:, :], in_=red_sb)
    if not nc.is_finalized():
        nc.finalize()
    return nc


def kernel(features_a, features_b, Wq1, Wq2, Wk1, Wk2, Wv1, Wv2):
    import time as _t

    features_a = np.asarray(features_a, dtype=np.float32)
    features_b = np.asarray(features_b, dtype=np.float32)
    fa = np.ascontiguousarray(features_a.reshape(B, C, S))
    fb = np.ascontiguousarray(features_b.reshape(B, C, S))

    if "nc" not in _CACHE:
        _CACHE["nc"] = _build_nc()

    w1 = np.stack([Wq1, Wk1, Wv1]).astype(np.float32).astype(NPBF).reshape(-1)
    w2 = np.stack([Wq2, Wk2, Wv2]).astype(np.float32).astype(NPBF).reshape(-1)
    wflat = np.frombuffer(
        np.ascontiguousarray(np.concatenate([w1, w2])).tobytes(), dtype=NPF8
    )

    in_maps = []
    for c in range(NCORES):
        sl = slice(c * BL, (c + 1) * BL)
        xa = fa[sl].transpose(1, 0, 2).reshape(C, NL)
        xb = fb[sl].transpose(1, 0, 2).reshape(C, NL)
        xT = np.concatenate([xa, xb], axis=1).astype(NPF8)
        in_maps.append({"x": xT, "wsh": wflat[c * WB:(c + 1) * WB].copy()})

    t0 = _t.time()
    res = run_bass_kernel_spmd(
        _CACHE["nc"], in_maps, list(range(NCORES)), trace=TRACE
    )
    LAST_EXEC_NS[0] = int((_t.time() - t0) * 1e9)

    sim = np.zeros((B, B), dtype=np.float64)
    for c in range(NCORES):
        o = res.results[c]["osim"].astype(np.float64).reshape(128, 2, BL)
        bidx = slice(c * BL, (c + 1) * BL)
        # dir ba: rows = global a, cols = local b
        sim[bidx, :] += o[:, 0, :].T
        # dir ab: rows = (p, cch, i), cols = aloc; b_local = 2p+i, a = cch*16+aloc
        ab = o[:, 1, :].reshape(8, 8, 2, BL)
        sim[bidx, :] += ab.transpose(0, 2, 1, 3).reshape(BL, B)
    return (sim / S).astype(np.float32)


# revision 23
# speedup vs baseline: 1.3610x; 1.0328x over previous
"""AttentionSimilarity Trainium2 kernel (8-core SPMD, single fused launch).

Host<->device traffic over the axon tunnel (~44 MB/s) dominates wall time,
so everything runs in ONE launch with minimal wire bytes:
  - features shipped bf16, fully sharded: core c gets its 16 a-batches and
    16 b-batches as an E-major [C, 1568] slab (2.35 MB/core).
  - projector weights shipped SHARDED 1/8 per core (0.5 MB/core) and
    AllGather-ed on device to the full 4 MB.
  - projections q/k/v computed on device; (qa, ka, va) AllGather-ed across
    cores (441 KB/core in, 3.5 MB out) so each core holds the full "a" side.
  - Gram matrices, norms, attention (exp-trick: softmax normalization
    cancels in cosine), cosine, and the mean over q all happen on device.
  - output per core: [128, 32] f32 (16 KB).

Attention structure per direction (from the two-launch baseline):
  scoresT = k_pair.T @ q            (PE)  e = exp(scale*scoresT)   (ACT)
  G = v_pair.T @ v_hat_other        (PE)  num = mask.T @ (e*G)     (PE)
  R = Gram_blockdiag.T @ e          (PE)  den = mask.T @ (e*R)     (PE)
  cos = num / sqrt(den)             (ACT+DVE, v_hat pre-normalized)
"""

import math

import ml_dtypes
import numpy as np

import concourse.bass as bass
from concourse import bacc
import concourse.mybir as mybir
from concourse.tile import TileContext
from concourse.bass_utils import run_bass_kernel_spmd

BF16 = mybir.dt.bfloat16
F32 = mybir.dt.float32
FP8 = mybir.dt.float8e4
NPBF = ml_dtypes.bfloat16
NPF8 = ml_dtypes.float8_e4m3

B = 128
C = 768
S = 49
E = 96
NCORES = 8
BL = B // NCORES          # 16 local batches
NL = BL * S               # 784 local rows
NROWS = 2 * NL            # 1568 rows per core (a slab then b slab)
KT = C // 128             # 6 contraction tiles
W1E = 3 * C * C           # 1769472
W2E = 3 * C * E           # 221184
WELEM = W1E + W2E         # 1990656
WSH = WELEM // NCORES     # 248832
SCALE = 1.0 / math.sqrt(E)
GRP = [list(range(NCORES))]

TRACE = False
LAST_EXEC_NS = [None, None]

_CACHE = {}


def _nchunks(total, step=512):
    out = []
    n0 = 0
    while n0 < total:
        out.append((n0, min(step, total - n0)))
        n0 += step
    return out


def _phase_b_projections(nc, tc, x, wg, ptp):
    """pT[e, w, n] = ([relu(x.T @ W1_w) @ W2_w]).T, f32 in SBUF.

    x: fp8 AP [C*NROWS]; wg: fp8 byte AP [2*WELEM] holding bf16 weights.
    """
    RELU = mybir.ActivationFunctionType.Relu
    pT = ptp.tile([E, 3, NROWS], F32, tag="pT")
    with (
        tc.tile_pool(name="xp", bufs=1) as xp,
        tc.tile_pool(name="xcp", bufs=2) as xcp,
        tc.tile_pool(name="wp", bufs=1) as wp,
        tc.tile_pool(name="hp", bufs=1) as hp,
        tc.tile_pool(name="pp1", bufs=4, space="PSUM") as pp1,
        tc.tile_pool(name="pp2", bufs=2, space="PSUM") as pp2,
    ):
        x_sb = xp.tile([128, KT, NROWS], FP8)
        nc.sync.dma_start(
            out=x_sb,
            in_=x.bitcast(FP8).rearrange(
                "(t p n) -> p t n", t=KT, p=128, n=NROWS
            ),
        )
        w1_sb = wp.tile([128, 3, KT, C], BF16, tag="w1")
        nc.sync.dma_start(
            out=w1_sb,
            in_=wg[0:W1E].rearrange(
                "(w t p n) -> p w t n", w=3, t=KT, p=128, n=C
            ),
        )
        w2_sb = wp.tile([128, 3, KT, E], BF16, tag="w2")
        nc.sync.dma_start(
            out=w2_sb,
            in_=wg[W1E:WELEM].rearrange(
                "(w t p n) -> p w t n", w=3, t=KT, p=128, n=E
            ),
        )

        for w in range(3):
            hT = hp.tile([128, KT, NROWS], BF16, tag="hT")
            for n0, nsz in _nchunks(NROWS):
                xc = xcp.tile([128, KT, 512], BF16, tag="xc")
                nc.scalar.copy(xc[:, :, :nsz], x_sb[:, :, n0:n0 + nsz])
                for m in range(KT):
                    ps = pp1.tile([128, 512], F32, tag="ps1")
                    for k in range(KT):
                        nc.tensor.matmul(
                            ps[:, :nsz],
                            lhsT=w1_sb[:, w, k, m * 128:(m + 1) * 128],
                            rhs=xc[:, k, :nsz],
                            start=(k == 0),
                            stop=(k == KT - 1),
                        )
                    nc.scalar.activation(hT[:, m, n0:n0 + nsz], ps[:, :nsz], RELU)
            for n0, nsz in _nchunks(NROWS):
                ps2 = pp2.tile([E, 512], F32, tag="ps2")
                for k in range(KT):
                    nc.tensor.matmul(
                        ps2[:, :nsz],
                        lhsT=w2_sb[:, w, k, :],
                        rhs=hT[:, k, n0:n0 + nsz],
                        start=(k == 0),
                        stop=(k == KT - 1),
                    )
                nc.scalar.copy(pT[:, w, n0:n0 + nsz], ps2[:, :nsz])
    return pT


XB = C * NROWS            # fp8 feature bytes per core
WB = 2 * WSH              # weight-shard bytes per core (bf16 as raw bytes)


def _build_nc():
    nc = bacc.Bacc(target_bir_lowering=False, num_devices=NCORES)
    xin = nc.declare_dram_parameter("xin", [XB // 2 + WSH], BF16, isOutput=False)
    osim = nc.declare_dram_parameter("osim", [128, 32], F32, isOutput=True)

    EXP = mybir.ActivationFunctionType.Exp
    SQRT = mybir.ActivationFunctionType.Sqrt
    BYP = mybir.AluOpType.bypass

    with TileContext(nc) as tc:
        with (
            tc.tile_pool(name="dram", bufs=1, space="DRAM") as dp,
            tc.tile_pool(name="cst", bufs=1) as cst,
        ):
            # ---- Phase A: weight shard AllGather (overlaps x load) ----
            wsh_b = dp.tile([WSH], BF16, tag="wshb")
            wg = dp.tile([WELEM], BF16, tag="wg")
            nc.gpsimd.dma_start(out=wsh_b, in_=xin[XB // 2:XB // 2 + WSH])
            nc.gpsimd.collective_compute(
                "AllGather", BYP, replica_groups=GRP,
                ins=[wsh_b[:].opt()], outs=[wg[:].opt()],
            )

            # persistent attention operands
            qb_sb = cst.tile([E, NL], BF16, tag="qb")
            vbh_sb = cst.tile([E, NL], BF16, tag="vbh")
            kbp_sb = cst.tile([E, 8, 128], BF16, tag="kbp")
            vbp_sb = cst.tile([E, 8, 128], BF16, tag="vbp")
            qa_sb = cst.tile([E, 8 * NL], BF16, tag="qa")
            vah_sb = cst.tile([E, 8 * NL], BF16, tag="vah")
            kap_sb = cst.tile([E, 64, 128], BF16, tag="kap")
            vap_sb = cst.tile([E, 64, 128], BF16, tag="vap")
            ma_sb = cst.tile([128, 64, 128], BF16, tag="ma")
            mb_sb = cst.tile([128, 8, 128], BF16, tag="mb")
            msk_sb = cst.tile([128, 256], BF16, tag="msk")
            cos_sb = cst.tile([128, 2, NL], F32, tag="cos")
            ones_c = cst.tile([E, 1], F32, tag="onec")
            ones_r = cst.tile([1, E], F32, tag="oner")

            nc.vector.memset(msk_sb[:], 0.0)
            nc.vector.memset(msk_sb[0:S, 126:127], 1.0)
            nc.vector.memset(msk_sb[64:64 + S, 127:128], 1.0)
            nc.vector.memset(ones_c[:], 1.0)
            nc.vector.memset(ones_r[:], 1.0)
            nc.vector.memset(kbp_sb[:], 0.0)
            nc.vector.memset(vbp_sb[:], 0.0)
            nc.vector.memset(kap_sb[:], 0.0)
            nc.vector.memset(vap_sb[:], 0.0)

            with tc.tile_pool(name="ptp", bufs=1) as ptp:
                # ---- Phase B: projections ----
                pT = _phase_b_projections(nc, tc, xin[0:XB // 2], wg, ptp)

                # ---- Phase C: gather (qa,ka,va) + local bf16 prep ----
                pg_sb = ptp.tile([E, 3, NL], BF16, tag="pg")
                for w in range(3):
                    nc.scalar.copy(pg_sb[:, w, :], pT[:, w, 0:NL])
                g_in = dp.tile([E, 3, NL], BF16, tag="gin")
                gout = dp.tile([NCORES * E * 3 * NL], BF16, tag="gout")
                nc.sync.dma_start(out=g_in, in_=pg_sb)
                nc.gpsimd.collective_compute(
                    "AllGather", BYP, replica_groups=GRP,
                    ins=[g_in[:].opt()], outs=[gout[:].opt()],
                )

                nc.scalar.copy(qb_sb[:], pT[:, 0, NL:NROWS])
                for w, dst in ((1, kbp_sb), (2, vbp_sb)):
                    src = pT[:, w, NL:NROWS].rearrange(
                        "p (b2 i s) -> p i b2 s", b2=8, i=2, s=S
                    )
                    for i in range(2):
                        nc.scalar.copy(dst[:, :, 64 * i:64 * i + S], src[:, i])

                with (
                    tc.tile_pool(name="np1", bufs=2) as np1,
                    tc.tile_pool(name="npp", bufs=2, space="PSUM") as npp,
                ):
                    def normalize(dst_ap, src_ap, nsz):
                        sq = np1.tile([E, 512], F32, tag="sq")
                        nc.vector.tensor_mul(sq[:, :nsz], src_ap, src_ap)
                        ssq = npp.tile([1, 512], F32, tag="ssq")
                        nc.tensor.matmul(
                            ssq[:, :nsz], lhsT=ones_c[:, :], rhs=sq[:, :nsz],
                            start=True, stop=True,
                        )
                        rno = np1.tile([1, 512], F32, tag="rno")
                        nc.scalar.activation(rno[:, :nsz], ssq[:, :nsz], SQRT)
                        rrec = np1.tile([1, 512], F32, tag="rrec")
                        nc.vector.reciprocal(rrec[:, :nsz], rno[:, :nsz])
                        bc = npp.tile([E, 512], F32, tag="bc")
                        nc.tensor.matmul(
                            bc[:, :nsz], lhsT=ones_r[:, :], rhs=rrec[:, :nsz],
                            start=True, stop=True,
                        )
                        nc.vector.tensor_mul(dst_ap, src_ap, bc[:, :nsz])

                    # vb_hat from local f32 vb
                    for n0, nsz in _nchunks(NL):
                        normalize(
                            vbh_sb[:, n0:n0 + nsz],
                            pT[:, 2, NL + n0:NL + n0 + nsz], nsz,
                        )

                    # gathered loads
                    gv = gout.rearrange(
                        "(c p w n) -> p w c n", c=NCORES, p=E, w=3, n=NL
                    )
                    nc.sync.dma_start(
                        out=qa_sb.rearrange("p (c n) -> p c n", c=NCORES, n=NL),
                        in_=gv[:, 0],
                    )
                    va_fl = ptp.tile([E, 8 * NL], BF16, tag="vafl")
                    nc.sync.dma_start(
                        out=va_fl.rearrange("p (c n) -> p c n", c=NCORES, n=NL),
                        in_=gv[:, 2],
                    )
                    gvp = gout.rearrange(
                        "(c p w b2 i s) -> p w i c b2 s",
                        c=NCORES, p=E, w=3, b2=8, i=2, s=S,
                    )
                    for i in range(2):
                        for cc in range(NCORES):
                            nc.sync.dma_start(
                                out=kap_sb[:, cc * 8:(cc + 1) * 8,
                                           64 * i:64 * i + S],
                                in_=gvp[:, 1, i, cc],
                            )
                            nc.sync.dma_start(
                                out=vap_sb[:, cc * 8:(cc + 1) * 8,
                                           64 * i:64 * i + S],
                                in_=gvp[:, 2, i, cc],
                            )

                    # va_hat from gathered bf16 va
                    for n0, nsz in _nchunks(8 * NL):
                        normalize(
                            vah_sb[:, n0:n0 + nsz], va_fl[:, n0:n0 + nsz], nsz
                        )

                # Gram matrices (blockdiag pair layout), from bf16 pads
                with tc.tile_pool(name="grm", bufs=4, space="PSUM") as grm:
                    nc.vector.memset(ma_sb[:], 0.0)
                    nc.vector.memset(mb_sb[:], 0.0)
                    for j in range(64):
                        pg = grm.tile([128, 128], F32, tag="g")
                        for i in range(2):
                            sl = slice(64 * i, 64 * i + S)
                            nc.tensor.matmul(
                                pg[sl, sl],
                                lhsT=vap_sb[:, j, sl], rhs=vap_sb[:, j, sl],
                                start=True, stop=True,
                            )
                            nc.scalar.copy(ma_sb[sl, j, sl], pg[sl, sl])
                    for p8 in range(8):
                        pg = grm.tile([128, 128], F32, tag="g")
                        for i in range(2):
                            sl = slice(64 * i, 64 * i + S)
                            nc.tensor.matmul(
                                pg[sl, sl],
                                lhsT=vbp_sb[:, p8, sl], rhs=vbp_sb[:, p8, sl],
                                start=True, stop=True,
                            )
                            nc.scalar.copy(mb_sb[sl, p8, sl], pg[sl, sl])

            # ---- Phase D: attention + cosine + q-sum ----
            with (
                tc.tile_pool(name="ep", bufs=6) as ep,
                tc.tile_pool(name="prp", bufs=6) as prp,
                tc.tile_pool(name="ep2", bufs=2) as ep2,
                tc.tile_pool(name="op", bufs=1) as op,
                tc.tile_pool(name="sgr", bufs=2, space="PSUM") as sgr,
                tc.tile_pool(name="grp", bufs=2, space="PSUM") as grp_ps,
                tc.tile_pool(name="ppd", bufs=1, space="PSUM") as ppd,
            ):
                chunks = _nchunks(NL)
                for d in range(2):
                    if d == 0:  # dir ba: a-pair j vs all local b
                        units = [
                            (
                                kap_sb[:, j, :],
                                vap_sb[:, j, :],
                                qb_sb,
                                vbh_sb,
                                ma_sb[:, j, :],
                            )
                            for j in range(64)
                        ]
                    else:  # dir ab: local b-pair p vs a-chunk cch
                        units = [
                            (
                                kbp_sb[:, p, :],
                                vbp_sb[:, p, :],
                                qa_sb[:, cch * NL:(cch + 1) * NL],
                                vah_sb[:, cch * NL:(cch + 1) * NL],
                                mb_sb[:, p, :],
                            )
                            for p in range(8)
                            for cch in range(8)
                        ]
                    for n0, nsz in chunks:
                        ps_num = ppd.tile([128, 512], F32, tag="dnum")
                        ps_den = ppd.tile([128, 512], F32, tag="dden")
                        for j, (lk, lv, rq, rv, mm) in enumerate(units):
                            mwin = msk_sb[:, 126 - 2 * j:254 - 2 * j]
                            ps_s = sgr.tile([128, 512], F32, tag="sgr")
                            nc.tensor.matmul(
                                ps_s[:, :nsz],
                                lhsT=lk,
                                rhs=rq[:, n0:n0 + nsz],
                                start=True,
                                stop=True,
                            )
                            eh = ep.tile([128, 512], BF16, tag="eh")
                            nc.scalar.activation(
                                eh[:, :nsz], ps_s[:, :nsz], EXP, scale=SCALE
                            )
                            ps_gr = grp_ps.tile([128, 2, 512], F32, tag="gr2")
                            nc.tensor.matmul(
                                ps_gr[:, 0, :nsz],
                                lhsT=lv,
                                rhs=rv[:, n0:n0 + nsz],
                                start=True,
                                stop=True,
                            )
                            nc.tensor.matmul(
                                ps_gr[:, 1, :nsz],
                                lhsT=mm,
                                rhs=eh[:, :nsz],
                                start=True,
                                stop=True,
                            )
                            pgr = prp.tile([128, 2, 512], BF16, tag="pgr")
                            eh2 = bass.AP(
                                tensor=eh.tensor,
                                offset=eh.offset,
                                ap=[eh.ap[0], [0, 2], [1, nsz]],
                            )
                            nc.vector.tensor_mul(
                                pgr[:, :, :nsz], eh2, ps_gr[:, :, :nsz]
                            )
                            nc.tensor.matmul(
                                ps_num[:, :nsz],
                                lhsT=mwin,
                                rhs=pgr[:, 0, :nsz],
                                start=(j == 0),
                                stop=(j == 63),
                            )
                            nc.tensor.matmul(
                                ps_den[:, :nsz],
                                lhsT=mwin,
                                rhs=pgr[:, 1, :nsz],
                                start=(j == 0),
                                stop=(j == 63),
                            )
                        sq_d = ep2.tile([128, 512], F32, tag="sqd")
                        nc.scalar.activation(sq_d[:, :nsz], ps_den[:, :nsz], SQRT)
                        rc_d = ep2.tile([128, 512], F32, tag="rcd")
                        nc.vector.reciprocal(rc_d[:, :nsz], sq_d[:, :nsz])
                        nc.vector.tensor_mul(
                            cos_sb[:, d, n0:n0 + nsz],
                            ps_num[:, :nsz],
                            rc_d[:, :nsz],
                        )

                red_sb = op.tile([128, 32], F32, tag="red")
                nc.vector.reduce_sum(
                    red_sb[:, :],
                    cos_sb.rearrange("p d (b s) -> p d b s", b=BL, s=S),
                    axis=mybir.AxisListType.X,
                )
                nc.sync.dma_start(out=osim[:, :], in_=red_sb)
    if not nc.is_finalized():
        nc.finalize()
    return nc


def kernel(features_a, features_b, Wq1, Wq2, Wk1, Wk2, Wv1, Wv2):
    import time as _t

    features_a = np.asarray(features_a, dtype=np.float32)
    features_b = np.asarray(features_b, dtype=np.float32)
    fa = np.ascontiguousarray(features_a.reshape(B, C, S))
    fb = np.ascontiguousarray(features_b.reshape(B, C, S))

    if "nc" not in _CACHE:
        _CACHE["nc"] = _build_nc()

    w1 = np.stack([Wq1, Wk1, Wv1]).astype(np.float32).astype(NPBF).reshape(-1)
    w2 = np.stack([Wq2, Wk2, Wv2]).astype(np.float32).astype(NPBF).reshape(-1)
    wflat = np.concatenate([w1, w2])

    in_maps = []
    for c in range(NCORES):
        sl = slice(c * BL, (c + 1) * BL)
        xa = fa[sl].transpose(1, 0, 2).reshape(C, NL)
        xb = fb[sl].transpose(1, 0, 2).reshape(C, NL)
        xT = np.concatenate([xa, xb], axis=1).astype(NPF8)
        xpk = np.frombuffer(np.ascontiguousarray(xT).tobytes(), dtype=NPBF)
        in_maps.append(
            {"xin": np.concatenate([xpk, wflat[c * WSH:(c + 1) * WSH]])}
        )

    t0 = _t.time()
    res = run_bass_kernel_spmd(
        _CACHE["nc"], in_maps, list(range(NCORES)), trace=TRACE
    )
    LAST_EXEC_NS[0] = int((_t.time() - t0) * 1e9)

    sim = np.zeros((B, B), dtype=np.float64)
    for c in range(NCORES):
        o = res.results[c]["osim"].astype(np.float64).reshape(128, 2, BL)
        bidx = slice(c * BL, (c + 1) * BL)
        # dir ba: rows = global a, cols = local b
        sim[bidx, :] += o[:, 0, :].T
        # dir ab: rows = (p, cch, i), cols = aloc; b_local = 2p+i, a = cch*16+aloc
        ab = o[:, 1, :].reshape(8, 8, 2, BL)
        sim[bidx, :] += ab.transpose(0, 2, 1, 3).reshape(BL, B)
    return (sim / S).astype(np.float32)
